# revision 19
# baseline (speedup 1.0000x reference)
"""BiasedMultiHeadAttention Trainium2 kernel (v2: row-tiled attention).

Sharding: 8 cores = (batch b, query-half qh). Each core computes the full
pipeline for its 512 query rows of batch b (K/V projections for the batch
are duplicated across the 2 cores sharing it). No collectives.

Device layout trick: per-core x rows are host-rolled so the core's query
block is always rows 0..511 -> one SPMD program for all 8 cores; bias/mask
are rolled consistently (softmax sum order irrelevant).

Math folding (host, exact):
  xn_aff = ln(x)*g + b folded into weights:  w_eff[i,o] = w[o,i]*ln_g[i]
  b_eff[o] = (w @ ln_b + b)[o];  Q scaled by SCALE. The per-head gated bias
  is pre-exponentiated on host: eb = exp(gate_h * bias); the device computes
  softmax numerator as exp(logits) * eb (one vector multiply, no bias add).

PE usage: attention runs in 64x128 row-tiled mode - head A uses array rows
0:64 (tile (0,0)), head B rows 64:128 (tile (64,0)); the two matmuls run
concurrently, doubling QK throughput (contract=64 per head) and AV
throughput (contract 128 split into two 64-halves accumulated in separate
PSUM banks, summed during the normalize pass).
"""

import numpy as np
import ml_dtypes

import concourse.bass as bass
import concourse.tile as tile
import concourse.mybir as mybir
from concourse import bacc
from concourse.bass_utils import run_bass_kernel_spmd
from concourse.masks import make_identity

B, L, E, H = 4, 1024, 1024, 16
D = E // H
SCALE = D**-0.5
EPS = 1e-5
NCORES = 8
QL = 512  # query rows per core
PT = 128  # partitions
NL = L // PT  # 8 l-chunks
NE = E // PT  # 8 e-chunks
HP = H // 2  # 8 head pairs

F32 = mybir.dt.float32
BF16 = mybir.dt.bfloat16
FP8 = mybir.dt.float8e4
I32 = mybir.dt.int32
BF_NP = ml_dtypes.bfloat16
FP8_NP = ml_dtypes.float8_e4m3
# host-side up-scales keeping fp8 weights ~N(0, 0.5); undone at psum evac
QSC, KSC, VSC = 64.0, 16.0, 16.0

LAST_RESULT = None  # BassKernelResults of the most recent run (for test.py)


def _build_nc(use_pbias, use_mask, dump=()):
    """Build the single-core Bass program (same NEFF for all 8 cores).

    use_pbias: 4 bools - include projection-bias rank-1 matmuls for q,k,v,o
    use_mask: include key/query mask handling
    dump: debug intermediate names to emit as extra outputs
    """
    nc = bacc.Bacc("TRN2", target_bir_lowering=False, debug=False)
    dump_d = {}

    def dump_tile(name, ap):
        if name in dump:
            d = nc.dram_tensor("d_" + name, list(ap.shape), ap.dtype,
                               kind="ExternalOutput")
            nc.sync.dma_start(d[tuple(slice(None) for _ in ap.shape)], ap)
            dump_d[name] = d

    x_d = nc.dram_tensor("xc", [PT, NL, L], F32, kind="ExternalInput")
    eb_d = nc.dram_tensor("ebc", [H, L, QL], FP8, kind="ExternalInput")
    wq_d = nc.dram_tensor("wqt", [PT, NE, E], FP8, kind="ExternalInput")
    wk_d = nc.dram_tensor("wkt", [PT, NE, E], FP8, kind="ExternalInput")
    wv_d = nc.dram_tensor("wvt", [PT, NE, E], FP8, kind="ExternalInput")
    wo_d = nc.dram_tensor("wot", [PT, NE, E], BF16, kind="ExternalInput")
    pb_d = {}
    for name, use in zip("qkvo", use_pbias):
        if use:
            pb_d[name] = nc.dram_tensor(f"b{name}e", [1, E], BF16,
                                        kind="ExternalInput")
    if use_mask:
        km_d = nc.dram_tensor("kmc", [PT, NL], F32, kind="ExternalInput")
        mq_d = nc.dram_tensor("mqc", [1, QL], F32, kind="ExternalInput")
    y_d = nc.dram_tensor("yc", [QL, E], F32, kind="ExternalOutput")

    with tile.TileContext(nc) as tc:
        with (
            tc.tile_pool(name="persist", bufs=1) as pp,
            tc.tile_pool(name="consts", bufs=1) as cp,
        ):
            # ---- constants ----
            ident = cp.tile([PT, PT], BF16)
            make_identity(nc, ident)
            ones_row = cp.tile([1, L], BF16)
            nc.vector.memset(ones_row, 1.0)
            eps_t = cp.tile([PT, 1], F32)
            nc.vector.memset(eps_t, EPS)
            if use_mask:
                km_sb = cp.tile([PT, NL], F32)
                nc.sync.dma_start(km_sb, km_d[:, :])
                mqb = cp.tile([64, QL], F32)
                nc.gpsimd.dma_start(mqb,
                                    mq_d[0:1, :].partition_broadcast(64))

            # ---- resident tensors ----
            x_sb = pp.tile([PT, NL, L], F32)
            for lt in range(NL):
                nc.sync.dma_start(x_sb[:, lt, :], x_d[:, lt, :])
            wo_sb = pp.tile([PT, NE, E], BF16)
            kT = pp.tile([PT, NE, L], BF16)   # K^T [e_k, k]; rows 0:64 head
            #                                   A of pair, 64:128 head B
            v3 = pp.tile([PT, NL, H, 65], BF16)  # V | ones col per head
            qT = pp.tile([PT, NE, QL], BF16)    # Q^T (scaled) [e_q, q]
            oT = pp.tile([PT, NE, QL], BF16)    # attnout^T (normalized)
            nc.vector.memset(v3[:, :, :, 64:65], 1.0)  # ones cols
            pbr = {}
            for name in pb_d:
                pbr[name] = cp.tile([1, E], BF16)
                nc.sync.dma_start(pbr[name], pb_d[name][:, :])

            # ================= Phase 1: LayerNorm + transpose ============
            with (
                tc.tile_pool(name="ln", bufs=3) as lp,
                tc.tile_pool(name="xnt", bufs=1) as xp,
                tc.tile_pool(name="pst", bufs=4, space="PSUM") as ptp,
            ):
                xnT = xp.tile([PT, NE, L], FP8)  # xn^T [e, l]
                for lt in range(NL):
                    xr = x_sb[:, lt, :].rearrange("p (s d) -> p s d", s=2)
                    stats = lp.tile([PT, 2, 6], F32, tag="stats")
                    for sg in range(2):
                        nc.vector.bn_stats(stats[:, sg, :], xr[:, sg, :])
                    mv = lp.tile([PT, 2], F32, tag="mv")
                    nc.vector.bn_aggr(mv, stats)
                    sd = lp.tile([PT, 1], F32, tag="sd")
                    nc.scalar.activation(sd, mv[:, 1:2],
                                         mybir.ActivationFunctionType.Sqrt,
                                         bias=eps_t)
                    rs = lp.tile([PT, 1], F32, tag="rs")
                    nc.vector.reciprocal(rs, sd)
                    nmr = lp.tile([PT, 1], F32, tag="nmr")  # -mu*rs
                    nc.vector.tensor_scalar(
                        out=nmr, in0=mv[:, 0:1], scalar1=rs, scalar2=-1.0,
                        op0=mybir.AluOpType.mult, op1=mybir.AluOpType.mult)
                    xnb = lp.tile([PT, L], BF16, tag="xnb")
                    nc.scalar.activation(
                        xnb, x_sb[:, lt, :],
                        mybir.ActivationFunctionType.Identity,
                        bias=nmr, scale=rs)
                    for g in range(2):
                        psT = ptp.tile([PT, 512], BF16, tag="psT")
                        for j in range(4):
                            et = g * 4 + j
                            nc.tensor.transpose(
                                psT[:, j * PT:(j + 1) * PT],
                                xnb[:, et * PT:(et + 1) * PT], ident)
                        dst = xnT[:, g * 4:(g + 1) * 4,
                                  lt * PT:(lt + 1) * PT]
                        src = psT.rearrange("p (j l) -> p j l", j=4)
                        if g == 0:
                            nc.scalar.copy(dst, src)
                        else:
                            nc.vector.tensor_copy(dst, src)

                dump_tile("xnT", xnT[:, :, :])

                # ================= Phase 2: projections ==================
                with (
                    tc.tile_pool(name="wt", bufs=1) as wtp,
                    tc.tile_pool(name="prj", bufs=4, space="PSUM") as prp,
                ):
                    wk_sb = wtp.tile([PT, NE, E], FP8)
                    nc.sync.dma_start(wk_sb, wk_d[:, :, :])
                    wq_sb = wtp.tile([PT, NE, E], FP8)
                    nc.sync.dma_start(wq_sb, wq_d[:, :, :])
                    wv_sb = wtp.tile([PT, NE, E], FP8)
                    nc.sync.dma_start(wv_sb, wv_d[:, :, :])
                    nc.sync.dma_start(wo_sb, wo_d[:, :, :])
                    DR = mybir.MatmulPerfMode.DoubleRow
                    NK2 = NE // 2  # contract in pairs of 128-chunks

                    # K^T and Q^T: lhsT = w chunk, rhs = xn^T chunk
                    for ot in range(NE):
                        osl = slice(ot * PT, (ot + 1) * PT)
                        for nh in range(2):  # l halves
                            nsl = slice(nh * 512, (nh + 1) * 512)
                            ps = prp.tile([PT, 512], F32, tag="pp")
                            for k2 in range(NK2):
                                ksl = slice(2 * k2, 2 * k2 + 2)
                                nc.tensor.matmul(
                                    ps, wk_sb[:, ksl, osl],
                                    xnT[:, ksl, nsl], start=(k2 == 0),
                                    stop=(k2 == NK2 - 1 and "k" not in pbr),
                                    perf_mode=DR)
                            if "k" in pbr:
                                nc.tensor.matmul(ps, pbr["k"][:, osl],
                                                 ones_row[:, 0:512],
                                                 start=False, stop=True)
                            nc.scalar.mul(kT[:, ot, nsl], ps, 1.0 / KSC)
                        # Q (first 512 rolled rows only)
                        psq = prp.tile([PT, 512], F32, tag="pp")
                        for k2 in range(NK2):
                            ksl = slice(2 * k2, 2 * k2 + 2)
                            nc.tensor.matmul(
                                psq, wq_sb[:, ksl, osl], xnT[:, ksl, 0:512],
                                start=(k2 == 0),
                                stop=(k2 == NK2 - 1 and "q" not in pbr),
                                perf_mode=DR)
                        if "q" in pbr:
                            nc.tensor.matmul(psq, pbr["q"][:, osl],
                                             ones_row[:, 0:512],
                                             start=False, stop=True)
                        nc.vector.tensor_scalar_mul(qT[:, ot, 0:512], psq,
                                                    1.0 / QSC)

                    # V natural layout: lhsT = xn^T chunk, rhs = w chunk
                    for lt in range(NL):
                        lsl = slice(lt * PT, (lt + 1) * PT)
                        for vh in range(2):
                            vsl = slice(vh * 512, (vh + 1) * 512)
                            psv = prp.tile([PT, 512], F32, tag="pp")
                            for k2 in range(NK2):
                                ksl = slice(2 * k2, 2 * k2 + 2)
                                nc.tensor.matmul(
                                    psv, xnT[:, ksl, lsl], wv_sb[:, ksl, vsl],
                                    start=(k2 == 0),
                                    stop=(k2 == NK2 - 1 and "v" not in pbr),
                                    perf_mode=DR)
                            if "v" in pbr:
                                nc.tensor.matmul(psv, ones_row[:, 0:PT],
                                                 pbr["v"][:, vsl],
                                                 start=False, stop=True)
                            nc.scalar.mul(
                                v3[:, lt, vh * 8:(vh + 1) * 8, 0:64],
                                psv.rearrange("p (h d) -> p h d", h=8),
                                1.0 / VSC)

            dump_tile("kT", kT[:, :, :])
            dump_tile("qT", qT[:, :, :])
            dump_tile("v3", v3[:, :, :, :])

            # ================= Phase 3: attention ========================
            # 64x128 row-tiled PE mode throughout: head A on tile (0,0)
            # (SBUF rows 0:64), head B on tile (64,0) (rows 64:128); pairs
            # of matmuls execute concurrently.
            with (
                tc.tile_pool(name="sc", bufs=1, space="PSUM") as scp,
                tc.tile_pool(name="av", bufs=1, space="PSUM") as avp,
                tc.tile_pool(name="bias", bufs=12) as bp,
                tc.tile_pool(name="expp", bufs=3) as sp,
                tc.tile_pool(name="attn", bufs=3) as ap,
                tc.tile_pool(name="nrm", bufs=2) as rcp,
                tc.tile_pool(name="oo", bufs=2) as oop,
                tc.tile_pool(name="recd", bufs=2, space="DRAM") as rdp,
            ):
                for t in range(HP):
                    hA, hB = 2 * t, 2 * t + 1
                    avA0 = avp.tile([65, QL], F32, tag="avA0")
                    avA1 = avp.tile([65, QL], F32, tag="avA1")
                    avB0 = avp.tile([65, QL], F32, tag="avB0")
                    avB1 = avp.tile([65, QL], F32, tag="avB1")
                    for c in range(NL):
                        csl = slice(c * PT, (c + 1) * PT)
                        ps = scp.tile([PT, 2, QL], F32, tag="ps", bufs=2)
                        nc.tensor.matmul(ps[:, 0, :], kT[0:64, t, csl],
                                         qT[0:64, t, :], start=True,
                                         stop=True, tile_position=(0, 0))
                        nc.tensor.matmul(ps[:, 1, :], kT[64:128, t, csl],
                                         qT[64:128, t, :], start=True,
                                         stop=True, tile_position=(64, 0))
                        ebt = bp.tile([PT, 2, QL], FP8, tag="ebt")
                        nc.sync.dma_start(
                            ebt, eb_d[hA:hB + 1, csl, :].rearrange(
                                "h p q -> p h q"))
                        es = sp.tile([PT, 2, QL], BF16, tag="es")
                        kmb = km_sb[:, c:c + 1] if use_mask else 0.0
                        nc.scalar.activation(
                            es.rearrange("p h q -> p (h q)"),
                            ps.rearrange("p h q -> p (h q)"),
                            mybir.ActivationFunctionType.Exp, bias=kmb)
                        at = ap.tile([PT, 2, QL], BF16, tag="at")
                        nc.vector.tensor_mul(
                            at.rearrange("p h q -> p (h q)"),
                            es.rearrange("p h q -> p (h q)"),
                            ebt.rearrange("p h q -> p (h q)"))
                        if t == 0 and c == 0:
                            dump_tile("at0", at[:, 0, :])
                            dump_tile("at1", at[:, 1, :])
                        st, sp_ = (c == 0), (c == NL - 1)
                        nc.tensor.matmul(avA0, v3[0:64, c, hA, :],
                                         at[0:64, 0, :], start=st, stop=sp_,
                                         tile_position=(0, 0))
                        nc.tensor.matmul(avA1, v3[64:128, c, hA, :],
                                         at[64:128, 0, :], start=st,
                                         stop=sp_, tile_position=(64, 0))
                        nc.tensor.matmul(avB0, v3[0:64, c, hB, :],
                                         at[0:64, 1, :], start=st, stop=sp_,
                                         tile_position=(0, 0))
                        nc.tensor.matmul(avB1, v3[64:128, c, hB, :],
                                         at[64:128, 1, :], start=st,
                                         stop=sp_, tile_position=(64, 0))
                    # normalize: sum halves -> bcast rowsum -> recip -> mul
                    # (DVE can't read two PSUM operands: bounce the T8
                    # halves through SBUF first)
                    av1s = rcp.tile([65, 2, QL], F32, tag="av1s")
                    nc.vector.tensor_copy(av1s[:, 0, :], avA1)
                    nc.vector.tensor_copy(av1s[:, 1, :], avB1)
                    avs = rcp.tile([65, 2, QL], F32, tag="avs")
                    nc.vector.tensor_add(avs[:, 0, :], avA0, av1s[:, 0, :])
                    nc.vector.tensor_add(avs[:, 1, :], avB0, av1s[:, 1, :])
                    # shift rowsum row (partition 64) to partition 0 via
                    # sbuf->sbuf DMA, then gpsimd-broadcast to 64 partitions
                    rsrow = rcp.tile([1, 2 * QL], F32, tag="rsrow")
                    nc.gpsimd.dma_start(
                        rsrow, avs[64:65, :, :].rearrange("p h q -> p (h q)"))
                    rbs = oop.tile([64, 2, QL], F32, tag="rbs")
                    nc.gpsimd.partition_broadcast(
                        rbs.rearrange("p h q -> p (h q)"), rsrow[0:1, :])
                    nc.vector.reciprocal_approx_fast(
                        out=rbs.rearrange("p h q -> p (h q)"),
                        in_=rbs.rearrange("p h q -> p (h q)"))
                    if use_mask:
                        for hi in range(2):
                            nc.vector.tensor_mul(rbs[:, hi, :],
                                                 rbs[:, hi, :], mqb)
                    if t == 0:
                        dump_tile("rbs0", rbs[:, 0, :])
                    nc.vector.tensor_mul(oT[0:64, t, :], avs[0:64, 0, :],
                                         rbs[:, 0, :])
                    ot_odd = oop.tile([64, QL], BF16, tag="oo")
                    nc.vector.tensor_mul(ot_odd, avs[0:64, 1, :],
                                         rbs[:, 1, :])
                    nc.sync.dma_start(oT[64:128, t, :], ot_odd)

            dump_tile("oT", oT[:, :, :])

            # ====== Phase 4: out-proj directly in [q, e] + residual ======
            # final[q,e] = sum_i oT[i,q] * woT[i,e]: lhsT = oT q-block slice
            # (stationary), rhs = woT chunk. No transposes needed.
            with (
                tc.tile_pool(name="fp", bufs=2, space="PSUM") as fpp,
                tc.tile_pool(name="yo", bufs=2) as yop,
            ):
                for qb in range(4):
                    qsl = slice(qb * PT, (qb + 1) * PT)
                    psf = fpp.tile([PT, E], F32, tag="pf")
                    for eh in range(2):
                        esl = slice(eh * 512, (eh + 1) * 512)
                        for ic in range(NE):
                            nc.tensor.matmul(
                                psf[:, esl], oT[:, ic, qsl],
                                wo_sb[:, ic, esl], start=(ic == 0),
                                stop=(ic == NE - 1 and "o" not in pbr))
                        if "o" in pbr:
                            nc.tensor.matmul(psf[:, esl],
                                             ones_row[0:1, 0:PT],
                                             pbr["o"][:, esl],
                                             start=False, stop=True)
                    y_sb = yop.tile([PT, E], F32, tag="y")
                    nc.vector.tensor_add(y_sb, psf, x_sb[:, qb, :])
                    nc.sync.dma_start(y_d[qsl, :], y_sb)
    return nc


def _prep_inputs(x, bias, mask, wq, bq, wk, bk, wv, bv, wo, bo, gate,
                 ln_g, ln_b):
    """Host-side folding + per-core sharding. Returns (in_maps, meta)."""
    gate = np.asarray(gate, np.float32)
    ln_g = np.asarray(ln_g, np.float32)
    ln_b = np.asarray(ln_b, np.float32)

    wqt = (np.asarray(wq).T * ln_g[:, None] * (SCALE * QSC)).astype(FP8_NP)
    wkt = (np.asarray(wk).T * ln_g[:, None] * KSC).astype(FP8_NP)
    wvt = (np.asarray(wv).T * ln_g[:, None] * VSC).astype(FP8_NP)
    wot = np.asarray(wo).T.astype(BF_NP)
    bqe = ((np.asarray(wq) @ ln_b + np.asarray(bq))
           * (SCALE * QSC)).astype(np.float32)
    bke = ((np.asarray(wk) @ ln_b + np.asarray(bk)) * KSC).astype(np.float32)
    bve = ((np.asarray(wv) @ ln_b + np.asarray(bv)) * VSC).astype(np.float32)
    boe = np.asarray(bo, np.float32)
    use_pbias = tuple(bool(np.any(b)) for b in (bqe, bke, bve, boe))

    mask = np.asarray(mask, np.int32)
    use_mask = not bool(np.all(mask == 1))

    def wfmt(w):  # [E_in, E_out] -> [128, 8, E]
        return np.ascontiguousarray(
            w.reshape(NE, PT, E).transpose(1, 0, 2))

    shared = {"wqt": wfmt(wqt), "wkt": wfmt(wkt), "wvt": wfmt(wvt),
              "wot": wfmt(wot)}
    for name, use, b in zip("qkvo", use_pbias, (bqe, bke, bve, boe)):
        if use:
            shared[f"b{name}e"] = b.reshape(1, E).astype(BF_NP)

    x = np.asarray(x, np.float32)
    bias = np.asarray(bias, np.float32)
    in_maps = []
    for c in range(NCORES):
        b_idx, qh = divmod(c, 2)
        q0 = qh * QL
        xr = np.roll(x[b_idx], -q0, axis=0)  # query block first
        m = {}
        m.update(shared)
        m["xc"] = np.ascontiguousarray(
            xr.reshape(NL, PT, L).transpose(1, 0, 2))
        bs = bias[b_idx][:, q0:q0 + QL, :]  # [H, QL, L]
        bs = np.roll(bs, -q0, axis=2)       # roll key axis
        eb = np.exp(gate[:, None, None] * bs)
        m["ebc"] = np.ascontiguousarray(eb.swapaxes(1, 2)).astype(FP8_NP)
        if use_mask:
            mr = np.roll(mask[b_idx], -q0)
            kmf = (-10000.0 * (1.0 - mr.astype(np.float32)))
            m["kmc"] = np.ascontiguousarray(
                kmf.reshape(NL, PT).T).astype(np.float32)
            m["mqc"] = mr[:QL].astype(np.float32).reshape(1, QL)
        in_maps.append(m)
    return in_maps, (use_pbias, use_mask)


def kernel(**inputs):
    global LAST_RESULT
    in_maps, (use_pbias, use_mask) = _prep_inputs(**inputs)
    nc = _build_nc(use_pbias, use_mask)
    if not nc.is_finalized():
        nc.finalize()
    res = run_bass_kernel_spmd(nc, in_maps, core_ids=list(range(NCORES)))
    LAST_RESULT = res
    out = np.empty((B, L, E), np.float32)
    for c in range(NCORES):
        b_idx, qh = divmod(c, 2)
        out[b_idx, qh * QL:(qh + 1) * QL, :] = res.results[c]["yc"]
    return out


# revision 20
# speedup vs baseline: 1.0787x; 1.0787x over previous
"""BiasedMultiHeadAttention Trainium2 kernel (v2: row-tiled attention).

Sharding: 8 cores = (batch b, query-half qh). Each core computes the full
pipeline for its 512 query rows of batch b (K/V projections for the batch
are duplicated across the 2 cores sharing it). No collectives.

Device layout trick: per-core x rows are host-rolled so the core's query
block is always rows 0..511 -> one SPMD program for all 8 cores; bias/mask
are rolled consistently (softmax sum order irrelevant).

Math folding (host, exact):
  xn_aff = ln(x)*g + b folded into weights:  w_eff[i,o] = w[o,i]*ln_g[i]
  b_eff[o] = (w @ ln_b + b)[o];  Q scaled by SCALE. The per-head gated bias
  is pre-exponentiated on host: eb = exp(gate_h * bias); the device computes
  softmax numerator as exp(logits) * eb (one vector multiply, no bias add).

PE usage: attention runs in 64x128 row-tiled mode - head A uses array rows
0:64 (tile (0,0)), head B rows 64:128 (tile (64,0)); the two matmuls run
concurrently, doubling QK throughput (contract=64 per head) and AV
throughput (contract 128 split into two 64-halves accumulated in separate
PSUM banks, summed during the normalize pass).
"""

import numpy as np
import ml_dtypes

import concourse.bass as bass
import concourse.tile as tile
import concourse.mybir as mybir
from concourse import bacc
from concourse.bass_utils import run_bass_kernel_spmd
from concourse.masks import make_identity

B, L, E, H = 4, 1024, 1024, 16
D = E // H
SCALE = D**-0.5
EPS = 1e-5
NCORES = 8
QL = 512  # query rows per core
PT = 128  # partitions
NL = L // PT  # 8 l-chunks
NE = E // PT  # 8 e-chunks
HP = H // 2  # 8 head pairs

F32 = mybir.dt.float32
BF16 = mybir.dt.bfloat16
FP8 = mybir.dt.float8e4
I32 = mybir.dt.int32
BF_NP = ml_dtypes.bfloat16
FP8_NP = ml_dtypes.float8_e4m3
# host-side up-scales keeping fp8 weights ~N(0, 0.5); undone at psum evac
QSC, KSC, VSC = 64.0, 16.0, 16.0

LAST_RESULT = None  # BassKernelResults of the most recent run (for test.py)


def _build_nc(use_pbias, use_mask, dump=()):
    """Build the single-core Bass program (same NEFF for all 8 cores).

    use_pbias: 4 bools - include projection-bias rank-1 matmuls for q,k,v,o
    use_mask: include key/query mask handling
    dump: debug intermediate names to emit as extra outputs
    """
    nc = bacc.Bacc("TRN2", target_bir_lowering=False, debug=False)
    dump_d = {}

    def dump_tile(name, ap):
        if name in dump:
            d = nc.dram_tensor("d_" + name, list(ap.shape), ap.dtype,
                               kind="ExternalOutput")
            nc.sync.dma_start(d[tuple(slice(None) for _ in ap.shape)], ap)
            dump_d[name] = d

    x_d = nc.dram_tensor("xc", [PT, NL, L], F32, kind="ExternalInput")
    eb_d = nc.dram_tensor("ebc", [H, L, QL], BF16, kind="ExternalInput")
    wq_d = nc.dram_tensor("wqt", [PT, NE, E], FP8, kind="ExternalInput")
    wk_d = nc.dram_tensor("wkt", [PT, NE, E], FP8, kind="ExternalInput")
    wv_d = nc.dram_tensor("wvt", [PT, NE, E], FP8, kind="ExternalInput")
    wo_d = nc.dram_tensor("wot", [PT, NE, E], BF16, kind="ExternalInput")
    pb_d = {}
    for name, use in zip("qkvo", use_pbias):
        if use:
            pb_d[name] = nc.dram_tensor(f"b{name}e", [1, E], BF16,
                                        kind="ExternalInput")
    if use_mask:
        km_d = nc.dram_tensor("kmc", [PT, NL], F32, kind="ExternalInput")
        mq_d = nc.dram_tensor("mqc", [1, QL], F32, kind="ExternalInput")
    y_d = nc.dram_tensor("yc", [QL, E], F32, kind="ExternalOutput")

    with tile.TileContext(nc) as tc:
        with (
            tc.tile_pool(name="persist", bufs=1) as pp,
            tc.tile_pool(name="consts", bufs=1) as cp,
        ):
            # ---- constants ----
            ident = cp.tile([PT, PT], BF16)
            make_identity(nc, ident)
            ones_row = cp.tile([1, L], BF16)
            nc.vector.memset(ones_row, 1.0)
            eps_t = cp.tile([PT, 1], F32)
            nc.vector.memset(eps_t, EPS)
            if use_mask:
                km_sb = cp.tile([PT, NL], F32)
                nc.sync.dma_start(km_sb, km_d[:, :])
                mqb = cp.tile([64, QL], F32)
                nc.gpsimd.dma_start(mqb,
                                    mq_d[0:1, :].partition_broadcast(64))

            # ---- resident tensors ----
            x_sb = pp.tile([PT, NL, L], F32)
            for lt in range(NL):
                nc.sync.dma_start(x_sb[:, lt, :], x_d[:, lt, :])
            wo_sb = pp.tile([PT, NE, E], BF16)
            kT = pp.tile([PT, NE, L], BF16)   # K^T [e_k, k]; rows 0:64 head
            #                                   A of pair, 64:128 head B
            v3 = pp.tile([PT, NL, H, 65], BF16)  # V | ones col per head
            qT = pp.tile([PT, NE, QL], BF16)    # Q^T (scaled) [e_q, q]
            oT = pp.tile([PT, NE, QL], BF16)    # attnout^T (normalized)
            nc.vector.memset(v3[:, :, :, 64:65], 1.0)  # ones cols
            pbr = {}
            for name in pb_d:
                pbr[name] = cp.tile([1, E], BF16)
                nc.sync.dma_start(pbr[name], pb_d[name][:, :])

            # ================= Phase 1: LayerNorm + transpose ============
            with (
                tc.tile_pool(name="ln", bufs=3) as lp,
                tc.tile_pool(name="xnt", bufs=1) as xp,
                tc.tile_pool(name="pst", bufs=4, space="PSUM") as ptp,
            ):
                xnT = xp.tile([PT, NE, L], FP8)  # xn^T [e, l]
                for lt in range(NL):
                    xr = x_sb[:, lt, :].rearrange("p (s d) -> p s d", s=2)
                    stats = lp.tile([PT, 2, 6], F32, tag="stats")
                    for sg in range(2):
                        nc.vector.bn_stats(stats[:, sg, :], xr[:, sg, :])
                    mv = lp.tile([PT, 2], F32, tag="mv")
                    nc.vector.bn_aggr(mv, stats)
                    sd = lp.tile([PT, 1], F32, tag="sd")
                    nc.scalar.activation(sd, mv[:, 1:2],
                                         mybir.ActivationFunctionType.Sqrt,
                                         bias=eps_t)
                    rs = lp.tile([PT, 1], F32, tag="rs")
                    nc.vector.reciprocal(rs, sd)
                    nmr = lp.tile([PT, 1], F32, tag="nmr")  # -mu*rs
                    nc.vector.tensor_scalar(
                        out=nmr, in0=mv[:, 0:1], scalar1=rs, scalar2=-1.0,
                        op0=mybir.AluOpType.mult, op1=mybir.AluOpType.mult)
                    xnb = lp.tile([PT, L], BF16, tag="xnb")
                    nc.scalar.activation(
                        xnb, x_sb[:, lt, :],
                        mybir.ActivationFunctionType.Identity,
                        bias=nmr, scale=rs)
                    for g in range(2):
                        psT = ptp.tile([PT, 512], BF16, tag="psT")
                        for j in range(4):
                            et = g * 4 + j
                            nc.tensor.transpose(
                                psT[:, j * PT:(j + 1) * PT],
                                xnb[:, et * PT:(et + 1) * PT], ident)
                        dst = xnT[:, g * 4:(g + 1) * 4,
                                  lt * PT:(lt + 1) * PT]
                        src = psT.rearrange("p (j l) -> p j l", j=4)
                        if g == 0:
                            nc.scalar.copy(dst, src)
                        else:
                            nc.vector.tensor_copy(dst, src)

                dump_tile("xnT", xnT[:, :, :])

                # ================= Phase 2: projections ==================
                with (
                    tc.tile_pool(name="wt", bufs=1) as wtp,
                    tc.tile_pool(name="prj", bufs=4, space="PSUM") as prp,
                ):
                    wk_sb = wtp.tile([PT, NE, E], FP8)
                    nc.sync.dma_start(wk_sb, wk_d[:, :, :])
                    wq_sb = wtp.tile([PT, NE, E], FP8)
                    nc.sync.dma_start(wq_sb, wq_d[:, :, :])
                    wv_sb = wtp.tile([PT, NE, E], FP8)
                    nc.sync.dma_start(wv_sb, wv_d[:, :, :])
                    nc.sync.dma_start(wo_sb, wo_d[:, :, :])
                    DR = mybir.MatmulPerfMode.DoubleRow
                    NK2 = NE // 2  # contract in pairs of 128-chunks

                    # K^T and Q^T: lhsT = w chunk, rhs = xn^T chunk
                    for ot in range(NE):
                        osl = slice(ot * PT, (ot + 1) * PT)
                        for nh in range(2):  # l halves
                            nsl = slice(nh * 512, (nh + 1) * 512)
                            ps = prp.tile([PT, 512], F32, tag="pp")
                            for k2 in range(NK2):
                                ksl = slice(2 * k2, 2 * k2 + 2)
                                nc.tensor.matmul(
                                    ps, wk_sb[:, ksl, osl],
                                    xnT[:, ksl, nsl], start=(k2 == 0),
                                    stop=(k2 == NK2 - 1 and "k" not in pbr),
                                    perf_mode=DR)
                            if "k" in pbr:
                                nc.tensor.matmul(ps, pbr["k"][:, osl],
                                                 ones_row[:, 0:512],
                                                 start=False, stop=True)
                            nc.scalar.mul(kT[:, ot, nsl], ps, 1.0 / KSC)
                        # Q (first 512 rolled rows only)
                        psq = prp.tile([PT, 512], F32, tag="pp")
                        for k2 in range(NK2):
                            ksl = slice(2 * k2, 2 * k2 + 2)
                            nc.tensor.matmul(
                                psq, wq_sb[:, ksl, osl], xnT[:, ksl, 0:512],
                                start=(k2 == 0),
                                stop=(k2 == NK2 - 1 and "q" not in pbr),
                                perf_mode=DR)
                        if "q" in pbr:
                            nc.tensor.matmul(psq, pbr["q"][:, osl],
                                             ones_row[:, 0:512],
                                             start=False, stop=True)
                        nc.vector.tensor_scalar_mul(qT[:, ot, 0:512], psq,
                                                    1.0 / QSC)

                    # V natural layout: lhsT = xn^T chunk, rhs = w chunk
                    for lt in range(NL):
                        lsl = slice(lt * PT, (lt + 1) * PT)
                        for vh in range(2):
                            vsl = slice(vh * 512, (vh + 1) * 512)
                            psv = prp.tile([PT, 512], F32, tag="pp")
                            for k2 in range(NK2):
                                ksl = slice(2 * k2, 2 * k2 + 2)
                                nc.tensor.matmul(
                                    psv, xnT[:, ksl, lsl], wv_sb[:, ksl, vsl],
                                    start=(k2 == 0),
                                    stop=(k2 == NK2 - 1 and "v" not in pbr),
                                    perf_mode=DR)
                            if "v" in pbr:
                                nc.tensor.matmul(psv, ones_row[:, 0:PT],
                                                 pbr["v"][:, vsl],
                                                 start=False, stop=True)
                            nc.scalar.mul(
                                v3[:, lt, vh * 8:(vh + 1) * 8, 0:64],
                                psv.rearrange("p (h d) -> p h d", h=8),
                                1.0 / VSC)

            dump_tile("kT", kT[:, :, :])
            dump_tile("qT", qT[:, :, :])
            dump_tile("v3", v3[:, :, :, :])

            # ================= Phase 3: attention ========================
            # 64x128 row-tiled PE mode throughout: head A on tile (0,0)
            # (SBUF rows 0:64), head B on tile (64,0) (rows 64:128); pairs
            # of matmuls execute concurrently.
            with (
                tc.tile_pool(name="sc", bufs=1, space="PSUM") as scp,
                tc.tile_pool(name="av", bufs=1, space="PSUM") as avp,
                tc.tile_pool(name="bias", bufs=12) as bp,
                tc.tile_pool(name="expp", bufs=3) as sp,
                tc.tile_pool(name="attn", bufs=3) as ap,
                tc.tile_pool(name="nrm", bufs=2) as rcp,
                tc.tile_pool(name="oo", bufs=2) as oop,
                tc.tile_pool(name="recd", bufs=2, space="DRAM") as rdp,
            ):
                for t in range(HP):
                    hA, hB = 2 * t, 2 * t + 1
                    avA0 = avp.tile([65, QL], F32, tag="avA0")
                    avA1 = avp.tile([65, QL], F32, tag="avA1")
                    avB0 = avp.tile([65, QL], F32, tag="avB0")
                    avB1 = avp.tile([65, QL], F32, tag="avB1")
                    for c in range(NL):
                        csl = slice(c * PT, (c + 1) * PT)
                        ps = scp.tile([PT, 2, QL], F32, tag="ps", bufs=2)
                        nc.tensor.matmul(ps[:, 0, :], kT[0:64, t, csl],
                                         qT[0:64, t, :], start=True,
                                         stop=True, tile_position=(0, 0))
                        nc.tensor.matmul(ps[:, 1, :], kT[64:128, t, csl],
                                         qT[64:128, t, :], start=True,
                                         stop=True, tile_position=(64, 0))
                        ebt = bp.tile([PT, 2, QL], BF16, tag="ebt")
                        nc.sync.dma_start(
                            ebt, eb_d[hA:hB + 1, csl, :].rearrange(
                                "h p q -> p h q"))
                        es = sp.tile([PT, 2, QL], BF16, tag="es")
                        kmb = km_sb[:, c:c + 1] if use_mask else 0.0
                        nc.scalar.activation(
                            es.rearrange("p h q -> p (h q)"),
                            ps.rearrange("p h q -> p (h q)"),
                            mybir.ActivationFunctionType.Exp, bias=kmb)
                        at = ap.tile([PT, 2, QL], BF16, tag="at")
                        nc.vector.tensor_mul(
                            at.rearrange("p h q -> p (h q)"),
                            es.rearrange("p h q -> p (h q)"),
                            ebt.rearrange("p h q -> p (h q)"))
                        if t == 0 and c == 0:
                            dump_tile("at0", at[:, 0, :])
                            dump_tile("at1", at[:, 1, :])
                        st, sp_ = (c == 0), (c == NL - 1)
                        nc.tensor.matmul(avA0, v3[0:64, c, hA, :],
                                         at[0:64, 0, :], start=st, stop=sp_,
                                         tile_position=(0, 0))
                        nc.tensor.matmul(avA1, v3[64:128, c, hA, :],
                                         at[64:128, 0, :], start=st,
                                         stop=sp_, tile_position=(64, 0))
                        nc.tensor.matmul(avB0, v3[0:64, c, hB, :],
                                         at[0:64, 1, :], start=st, stop=sp_,
                                         tile_position=(0, 0))
                        nc.tensor.matmul(avB1, v3[64:128, c, hB, :],
                                         at[64:128, 1, :], start=st,
                                         stop=sp_, tile_position=(64, 0))
                    # normalize: sum halves -> bcast rowsum -> recip -> mul
                    # (DVE can't read two PSUM operands: bounce the T8
                    # halves through SBUF first)
                    av1s = rcp.tile([65, 2, QL], F32, tag="av1s")
                    nc.vector.tensor_copy(av1s[:, 0, :], avA1)
                    nc.vector.tensor_copy(av1s[:, 1, :], avB1)
                    avs = rcp.tile([65, 2, QL], F32, tag="avs")
                    nc.vector.tensor_add(avs[:, 0, :], avA0, av1s[:, 0, :])
                    nc.vector.tensor_add(avs[:, 1, :], avB0, av1s[:, 1, :])
                    # shift rowsum row (partition 64) to partition 0 via
                    # sbuf->sbuf DMA, then gpsimd-broadcast to 64 partitions
                    rsrow = rcp.tile([1, 2 * QL], F32, tag="rsrow")
                    nc.gpsimd.dma_start(
                        rsrow, avs[64:65, :, :].rearrange("p h q -> p (h q)"))
                    rbs = oop.tile([64, 2, QL], F32, tag="rbs")
                    nc.gpsimd.partition_broadcast(
                        rbs.rearrange("p h q -> p (h q)"), rsrow[0:1, :])
                    nc.vector.reciprocal_approx_fast(
                        out=rbs.rearrange("p h q -> p (h q)"),
                        in_=rbs.rearrange("p h q -> p (h q)"))
                    if use_mask:
                        for hi in range(2):
                            nc.vector.tensor_mul(rbs[:, hi, :],
                                                 rbs[:, hi, :], mqb)
                    if t == 0:
                        dump_tile("rbs0", rbs[:, 0, :])
                    nc.vector.tensor_mul(oT[0:64, t, :], avs[0:64, 0, :],
                                         rbs[:, 0, :])
                    ot_odd = oop.tile([64, QL], BF16, tag="oo")
                    nc.vector.tensor_mul(ot_odd, avs[0:64, 1, :],
                                         rbs[:, 1, :])
                    nc.sync.dma_start(oT[64:128, t, :], ot_odd)

            dump_tile("oT", oT[:, :, :])

            # ====== Phase 4: out-proj directly in [q, e] + residual ======
            # final[q,e] = sum_i oT[i,q] * woT[i,e]: lhsT = oT q-block slice
            # (stationary), rhs = woT chunk. No transposes needed.
            with (
                tc.tile_pool(name="fp", bufs=2, space="PSUM") as fpp,
                tc.tile_pool(name="yo", bufs=2) as yop,
            ):
                for qb in range(4):
                    qsl = slice(qb * PT, (qb + 1) * PT)
                    psf = fpp.tile([PT, E], F32, tag="pf")
                    for eh in range(2):
                        esl = slice(eh * 512, (eh + 1) * 512)
                        for ic in range(NE):
                            nc.tensor.matmul(
                                psf[:, esl], oT[:, ic, qsl],
                                wo_sb[:, ic, esl], start=(ic == 0),
                                stop=(ic == NE - 1 and "o" not in pbr))
                        if "o" in pbr:
                            nc.tensor.matmul(psf[:, esl],
                                             ones_row[0:1, 0:PT],
                                             pbr["o"][:, esl],
                                             start=False, stop=True)
                    y_sb = yop.tile([PT, E], F32, tag="y")
                    nc.vector.tensor_add(y_sb, psf, x_sb[:, qb, :])
                    nc.sync.dma_start(y_d[qsl, :], y_sb)
    return nc


def _prep_inputs(x, bias, mask, wq, bq, wk, bk, wv, bv, wo, bo, gate,
                 ln_g, ln_b):
    """Host-side folding + per-core sharding. Returns (in_maps, meta)."""
    gate = np.asarray(gate, np.float32)
    ln_g = np.asarray(ln_g, np.float32)
    ln_b = np.asarray(ln_b, np.float32)

    wqt = (np.asarray(wq).T * ln_g[:, None] * (SCALE * QSC)).astype(FP8_NP)
    wkt = (np.asarray(wk).T * ln_g[:, None] * KSC).astype(FP8_NP)
    wvt = (np.asarray(wv).T * ln_g[:, None] * VSC).astype(FP8_NP)
    wot = np.asarray(wo).T.astype(BF_NP)
    bqe = ((np.asarray(wq) @ ln_b + np.asarray(bq))
           * (SCALE * QSC)).astype(np.float32)
    bke = ((np.asarray(wk) @ ln_b + np.asarray(bk)) * KSC).astype(np.float32)
    bve = ((np.asarray(wv) @ ln_b + np.asarray(bv)) * VSC).astype(np.float32)
    boe = np.asarray(bo, np.float32)
    use_pbias = tuple(bool(np.any(b)) for b in (bqe, bke, bve, boe))

    mask = np.asarray(mask, np.int32)
    use_mask = not bool(np.all(mask == 1))

    def wfmt(w):  # [E_in, E_out] -> [128, 8, E]
        return np.ascontiguousarray(
            w.reshape(NE, PT, E).transpose(1, 0, 2))

    shared = {"wqt": wfmt(wqt), "wkt": wfmt(wkt), "wvt": wfmt(wvt),
              "wot": wfmt(wot)}
    for name, use, b in zip("qkvo", use_pbias, (bqe, bke, bve, boe)):
        if use:
            shared[f"b{name}e"] = b.reshape(1, E).astype(BF_NP)

    x = np.asarray(x, np.float32)
    bias = np.asarray(bias, np.float32)
    in_maps = []
    for c in range(NCORES):
        b_idx, qh = divmod(c, 2)
        q0 = qh * QL
        xr = np.roll(x[b_idx], -q0, axis=0)  # query block first
        m = {}
        m.update(shared)
        m["xc"] = np.ascontiguousarray(
            xr.reshape(NL, PT, L).transpose(1, 0, 2))
        bs = bias[b_idx][:, q0:q0 + QL, :]  # [H, QL, L]
        bs = np.roll(bs, -q0, axis=2)       # roll key axis
        eb = np.exp(gate[:, None, None] * bs)
        m["ebc"] = np.ascontiguousarray(eb.swapaxes(1, 2)).astype(BF_NP)
        if use_mask:
            mr = np.roll(mask[b_idx], -q0)
            kmf = (-10000.0 * (1.0 - mr.astype(np.float32)))
            m["kmc"] = np.ascontiguousarray(
                kmf.reshape(NL, PT).T).astype(np.float32)
            m["mqc"] = mr[:QL].astype(np.float32).reshape(1, QL)
        in_maps.append(m)
    return in_maps, (use_pbias, use_mask)


def kernel(**inputs):
    global LAST_RESULT
    in_maps, (use_pbias, use_mask) = _prep_inputs(**inputs)
    nc = _build_nc(use_pbias, use_mask)
    if not nc.is_finalized():
        nc.finalize()
    res = run_bass_kernel_spmd(nc, in_maps, core_ids=list(range(NCORES)))
    LAST_RESULT = res
    out = np.empty((B, L, E), np.float32)
    for c in range(NCORES):
        b_idx, qh = divmod(c, 2)
        out[b_idx, qh * QL:(qh + 1) * QL, :] = res.results[c]["yc"]
    return out


# revision 29
# speedup vs baseline: 1.1239x; 1.0419x over previous
"""BiasedMultiHeadAttention Trainium2 kernel (v5: attention-shadowed projections).

Sharding: 8 cores = (batch b, query-half qh). Each core computes the full
pipeline for its 512 query rows of batch b (K/V projections for the batch
are duplicated across the 2 cores sharing it). No collectives.

Device layout trick: per-core x rows are host-rolled so the core's query
block is always rows 0..511 -> one SPMD program for all 8 cores; bias/mask
are rolled consistently (softmax sum order irrelevant).

Math folding (host, exact):
  LN affine folded into weights; Q scaled by SCALE. The per-head gated bias
  is pre-exponentiated on host: eb = exp(gate_h * bias); the device computes
  softmax numerators as exp(logits) * eb (one vector multiply, no bias add).

Structure: the softmax exp on the Scalar engine (64 x ~1.1us) is the
irreducible critical path. Q/K/V projections (fp8 DoubleRow, 2x PE rate)
are streamed as PE filler inside the attention window so the whole kernel
rides the exp cadence: per head-pair t the c-loop runs QK -> exp -> *eb ->
AV while interleaving the projection matmuls for head-pair t+1. O-proj
(bf16) accumulates during the last pair and finishes after it.

PSUM budget (8 banks): QK ps [128,2,512] x2 bufs = 4, AV avA/avB [128,512]
= 2, projections pp [128,512] x2 bufs = 2. V is padded to 128 columns
(64 ch | ones | zeros) so AV stationaries are full 128-wide (FWL).
"""

import numpy as np
import ml_dtypes

import concourse.bass as bass
import concourse.tile as tile
import concourse.mybir as mybir
from concourse import bacc
from concourse.bass_utils import run_bass_kernel_spmd
from concourse.masks import make_identity

B, L, E, H = 4, 1024, 1024, 16
D = E // H
SCALE = D**-0.5
EPS = 1e-5
NCORES = 8
QL = 512  # query rows per core
PT = 128  # partitions
NL = L // PT  # 8 l-chunks
NE = E // PT  # 8 e-chunks
HP = H // 2  # 8 head pairs

F32 = mybir.dt.float32
BF16 = mybir.dt.bfloat16
FP8 = mybir.dt.float8e4
I32 = mybir.dt.int32
BF_NP = ml_dtypes.bfloat16
FP8_NP = ml_dtypes.float8_e4m3
# host-side up-scales keeping fp8 weights ~N(0, 0.5); undone at psum evac
QSC, KSC, VSC = 64.0, 16.0, 16.0

LAST_RESULT = None  # BassKernelResults of the most recent run (for test.py)


def _build_nc(use_pbias, use_mask, dump=()):
    """Build the single-core Bass program (same NEFF for all 8 cores)."""
    nc = bacc.Bacc("TRN2", target_bir_lowering=False, debug=False)
    dump_d = {}

    def dump_tile(name, ap):
        if name in dump:
            d = nc.dram_tensor("d_" + name, list(ap.shape), ap.dtype,
                               kind="ExternalOutput")
            nc.sync.dma_start(d[tuple(slice(None) for _ in ap.shape)], ap)
            dump_d[name] = d

    x_d = nc.dram_tensor("xc", [PT, NL, L], F32, kind="ExternalInput")
    eb_d = nc.dram_tensor("ebc", [H, L, QL], BF16, kind="ExternalInput")
    wq_d = nc.dram_tensor("wqt", [PT, NE, E], FP8, kind="ExternalInput")
    wk_d = nc.dram_tensor("wkt", [PT, NE, E], FP8, kind="ExternalInput")
    wv_d = nc.dram_tensor("wvt", [PT, NE, E], FP8, kind="ExternalInput")
    wo_d = nc.dram_tensor("wot", [PT, NE, E], BF16, kind="ExternalInput")
    pb_d = {}
    for name, use in zip("qkvo", use_pbias):
        if use:
            pb_d[name] = nc.dram_tensor(f"b{name}e", [1, E], BF16,
                                        kind="ExternalInput")
    if use_mask:
        km_d = nc.dram_tensor("kmc", [PT, NL], F32, kind="ExternalInput")
        mq_d = nc.dram_tensor("mqc", [1, QL], F32, kind="ExternalInput")
    y_d = nc.dram_tensor("yc", [QL, E], F32, kind="ExternalOutput")

    DR = mybir.MatmulPerfMode.DoubleRow
    NK2 = NE // 2  # fp8 DoubleRow contracts pairs of 128-chunks

    with tile.TileContext(nc) as tc:
        with (
            tc.tile_pool(name="persist", bufs=1) as pp,
            tc.tile_pool(name="consts", bufs=1) as cp,
        ):
            # ---- constants ----
            ident = cp.tile([PT, PT], BF16)
            make_identity(nc, ident)
            if any(use_pbias):
                ones_row = cp.tile([1, L], BF16)
                nc.vector.memset(ones_row, 1.0)
            eps_t = cp.tile([PT, 1], F32)
            nc.vector.memset(eps_t, EPS)
            if use_mask:
                km_sb = cp.tile([PT, NL], F32)
                nc.sync.dma_start(km_sb, km_d[:, :])
                mqb = cp.tile([64, QL], F32)
                nc.gpsimd.dma_start(mqb,
                                    mq_d[0:1, :].partition_broadcast(64))

            # ---- resident tensors ----
            x_sb = pp.tile([PT, NL, L], F32)
            for lt in range(NL):
                nc.sync.dma_start(x_sb[:, lt, :], x_d[:, lt, :])
            wo_sb = pp.tile([PT, NE, E], BF16)
            # K^T zero-padded per head parity (full-128 QK contracts with
            # the other head's rows zeroed; keeps FWL on the weight path)
            kTzA = pp.tile([PT, NE, L], BF16)
            kTzB = pp.tile([PT, NE, L], BF16)
            nc.gpsimd.memset(kTzA[64:128, :, :], 0.0)
            nc.gpsimd.memset(kTzB[0:64, :, :], 0.0)
            # V padded to 128 cols per head: [ch(64) | ones | zeros(63)]
            v4 = pp.tile([PT, NL, H, PT], BF16)
            nc.gpsimd.memset(v4[:, :, :, 64:128], 0.0)
            nc.vector.memset(v4[:, :, :, 64:65], 1.0)
            qT = pp.tile([PT, NE, QL], BF16)    # Q^T (scaled) [e_q, q]
            oT = pp.tile([PT, NE, QL], BF16)    # attnout^T (normalized)
            pbr = {}
            for name in pb_d:
                pbr[name] = cp.tile([1, E], BF16)
                nc.sync.dma_start(pbr[name], pb_d[name][:, :])

            with (
                tc.tile_pool(name="xnt", bufs=1) as xp,
                tc.tile_pool(name="wt", bufs=1) as wtp,
            ):
                xnT = xp.tile([PT, NE, L], FP8)  # xn^T [e, l]
                wk_sb = wtp.tile([PT, NE, E], FP8)
                nc.sync.dma_start(wk_sb, wk_d[:, :, :])
                wq_sb = wtp.tile([PT, NE, E], FP8)
                nc.sync.dma_start(wq_sb, wq_d[:, :, :])
                wv_sb = wtp.tile([PT, NE, E], FP8)
                nc.sync.dma_start(wv_sb, wv_d[:, :, :])
                nc.sync.dma_start(wo_sb, wo_d[:, :, :])

                with (
                    tc.tile_pool(name="prj", bufs=2, space="PSUM") as prp,
                ):
                    # ---------- projection helpers (filler units) --------
                    def proj_k(ot):
                        osl = slice(ot * PT, (ot + 1) * PT)
                        for nh in range(2):
                            nsl = slice(nh * 512, (nh + 1) * 512)
                            ps = prp.tile([PT, 512], F32, tag="pp",
                                          name=f"psk{ot}{nh}")
                            for k2 in range(NK2):
                                ksl = slice(2 * k2, 2 * k2 + 2)
                                yield nc.tensor.matmul(
                                    ps, wk_sb[:, ksl, osl], xnT[:, ksl, nsl],
                                    start=(k2 == 0),
                                    stop=(k2 == NK2 - 1 and "k" not in pbr),
                                    perf_mode=DR)
                            if "k" in pbr:
                                yield nc.tensor.matmul(
                                    ps, pbr["k"][:, osl], ones_row[:, 0:512],
                                    start=False, stop=True)
                            yield nc.scalar.mul(kTzA[0:64, ot, nsl],
                                                ps[0:64, :], 1.0 / KSC)
                            yield nc.vector.tensor_scalar_mul(
                                kTzB[64:128, ot, nsl], ps[64:128, :],
                                1.0 / KSC)

                    def proj_q(ot):
                        osl = slice(ot * PT, (ot + 1) * PT)
                        psq = prp.tile([PT, 512], F32, tag="pp",
                                       name=f"psq{ot}")
                        for k2 in range(NK2):
                            ksl = slice(2 * k2, 2 * k2 + 2)
                            yield nc.tensor.matmul(
                                psq, wq_sb[:, ksl, osl], xnT[:, ksl, 0:512],
                                start=(k2 == 0),
                                stop=(k2 == NK2 - 1 and "q" not in pbr),
                                perf_mode=DR)
                        if "q" in pbr:
                            yield nc.tensor.matmul(
                                psq, pbr["q"][:, osl], ones_row[:, 0:512],
                                start=False, stop=True)
                        yield nc.vector.tensor_scalar_mul(qT[:, ot, 0:512],
                                                          psq, 1.0 / QSC)

                    def proj_v(lt, vh):
                        lsl = slice(lt * PT, (lt + 1) * PT)
                        vsl = slice(vh * 512, (vh + 1) * 512)
                        psv = prp.tile([PT, 512], F32, tag="pp",
                                       name=f"psv{lt}{vh}")
                        for k2 in range(NK2):
                            ksl = slice(2 * k2, 2 * k2 + 2)
                            yield nc.tensor.matmul(
                                psv, xnT[:, ksl, lsl], wv_sb[:, ksl, vsl],
                                start=(k2 == 0),
                                stop=(k2 == NK2 - 1 and "v" not in pbr),
                                perf_mode=DR)
                        if "v" in pbr:
                            yield nc.tensor.matmul(
                                psv, ones_row[:, 0:PT], pbr["v"][:, vsl],
                                start=False, stop=True)
                        yield nc.scalar.mul(
                            v4[:, lt, vh * 8:(vh + 1) * 8, 0:64],
                            psv.rearrange("p (h d) -> p h d", h=8),
                            1.0 / VSC)

                    # ========== Phase 1: LayerNorm + transpose + V(vh0) ==
                    lp = tc.alloc_tile_pool(name="ln", bufs=3)
                    ptp = tc.alloc_tile_pool(name="pst", bufs=2,
                                             space="PSUM")
                    for lt in range(NL):
                        xr = x_sb[:, lt, :].rearrange("p (s d) -> p s d",
                                                      s=2)
                        stats = lp.tile([PT, 2, 6], F32, tag="stats")
                        for sg in range(2):
                            nc.vector.bn_stats(stats[:, sg, :], xr[:, sg, :])
                        mv = lp.tile([PT, 2], F32, tag="mv")
                        nc.vector.bn_aggr(mv, stats)
                        sd = lp.tile([PT, 1], F32, tag="sd")
                        nc.scalar.activation(
                            sd, mv[:, 1:2],
                            mybir.ActivationFunctionType.Sqrt, bias=eps_t)
                        rs = lp.tile([PT, 1], F32, tag="rs")
                        nc.vector.reciprocal(rs, sd)
                        nmr = lp.tile([PT, 1], F32, tag="nmr")  # -mu*rs
                        nc.vector.tensor_scalar(
                            out=nmr, in0=mv[:, 0:1], scalar1=rs,
                            scalar2=-1.0, op0=mybir.AluOpType.mult,
                            op1=mybir.AluOpType.mult)
                        xnb = lp.tile([PT, L], BF16, tag="xnb")
                        nc.scalar.activation(
                            xnb, x_sb[:, lt, :],
                            mybir.ActivationFunctionType.Identity,
                            bias=nmr, scale=rs)
                        for g in range(2):
                            psT = ptp.tile([PT, 512], BF16, tag="psT")
                            for j in range(4):
                                et = g * 4 + j
                                nc.tensor.transpose(
                                    psT[:, j * PT:(j + 1) * PT],
                                    xnb[:, et * PT:(et + 1) * PT], ident)
                            dst = xnT[:, g * 4:(g + 1) * 4,
                                      lt * PT:(lt + 1) * PT]
                            src = psT.rearrange("p (j l) -> p j l", j=4)
                            if g == 0:
                                nc.scalar.copy(dst, src)
                            else:
                                nc.vector.tensor_copy(dst, src)
                        # V(lt, vh=0) right behind this lt's transposes
                        for _ in proj_v(lt, 0):
                            pass

                    ptp.release()
                    lp.release()
                    scp = tc.alloc_tile_pool(name="sc", bufs=2,
                                             space="PSUM")
                    avp = tc.alloc_tile_pool(name="av", bufs=1,
                                             space="PSUM")
                    bp = tc.alloc_tile_pool(name="bias", bufs=6)
                    sp = tc.alloc_tile_pool(name="expp", bufs=3)
                    ap = tc.alloc_tile_pool(name="attn", bufs=3)
                    rcp = tc.alloc_tile_pool(name="nrm", bufs=2)
                    oop = tc.alloc_tile_pool(name="oo", bufs=2)
                    yop = tc.alloc_tile_pool(name="yo", bufs=2)

                    dump_tile("xnT", xnT[:, :, :])

                    # K/Q for head pair 0 (rest streams in the shadow)
                    for _ in proj_k(0):
                        pass
                    for _ in proj_q(0):
                        pass

                    # filler stream: K/Q for pairs 1..7, V vh=1 for all lt
                    def filler_ops():
                        for t1 in range(1, HP):
                            yield from proj_k(t1)
                            yield from proj_q(t1)
                            if t1 <= 4:
                                for lt in (2 * (t1 - 1), 2 * t1 - 1):
                                    yield from proj_v(lt, 1)
                    filler = filler_ops()
                    FILLER_PER_C = 4

                    # ================= attention ======================
                    for t in range(HP):
                        hA, hB = 2 * t, 2 * t + 1
                        avA = avp.tile([PT, QL], F32, tag="avA")
                        avB = avp.tile([PT, QL], F32, tag="avB")
                        for c in range(NL):
                            csl = slice(c * PT, (c + 1) * PT)
                            ps = scp.tile([PT, 2, QL], F32, tag="ps")
                            nc.tensor.matmul(ps[:, 0, :], kTzA[:, t, csl],
                                             qT[:, t, :], start=True,
                                             stop=True)
                            nc.tensor.matmul(ps[:, 1, :], kTzB[:, t, csl],
                                             qT[:, t, :], start=True,
                                             stop=True)
                            ebt = bp.tile([PT, 2, QL], BF16, tag="ebt")
                            nc.sync.dma_start(
                                ebt, eb_d[hA:hB + 1, csl, :].rearrange(
                                    "h p q -> p h q"))
                            es = sp.tile([PT, 2, QL], BF16, tag="es")
                            kmb = km_sb[:, c:c + 1] if use_mask else 0.0
                            nc.scalar.activation(
                                es.rearrange("p h q -> p (h q)"),
                                ps.rearrange("p h q -> p (h q)"),
                                mybir.ActivationFunctionType.Exp, bias=kmb)
                            at = ap.tile([PT, 2, QL], BF16, tag="at")
                            nc.vector.tensor_mul(
                                at.rearrange("p h q -> p (h q)"),
                                es.rearrange("p h q -> p (h q)"),
                                ebt.rearrange("p h q -> p (h q)"))
                            if t == 0 and c == 0:
                                dump_tile("at0", at[:, 0, :])
                                dump_tile("at1", at[:, 1, :])
                            st, sp_ = (c == 0), (c == NL - 1)
                            nc.tensor.matmul(avA, v4[:, c, hA, :],
                                             at[:, 0, :], start=st,
                                             stop=sp_)
                            nc.tensor.matmul(avB, v4[:, c, hB, :],
                                             at[:, 1, :], start=st,
                                             stop=sp_)
                            for _ in range(FILLER_PER_C):
                                if next(filler, None) is None:
                                    break
                        # ---- normalize ----
                        rsb = rcp.tile([65, 2, QL], F32, tag="rsb")
                        nc.vector.tensor_copy(rsb[64:65, 0, :],
                                              avA[64:65, :])
                        nc.vector.tensor_copy(rsb[64:65, 1, :],
                                              avB[64:65, :])
                        nc.gpsimd.dma_start(
                            rsb[0:1, :, :].rearrange("p h q -> p (h q)"),
                            rsb[64:65, :, :].rearrange("p h q -> p (h q)"))
                        rbs = oop.tile([64, 2, QL], F32, tag="rbs")
                        nc.gpsimd.partition_broadcast(
                            rbs.rearrange("p h q -> p (h q)"),
                            rsb[0:1, :, :].rearrange("p h q -> p (h q)"))
                        nc.vector.reciprocal_approx_fast(
                            out=rbs.rearrange("p h q -> p (h q)"),
                            in_=rbs.rearrange("p h q -> p (h q)"))
                        if use_mask:
                            for hi in range(2):
                                nc.vector.tensor_mul(rbs[:, hi, :],
                                                     rbs[:, hi, :], mqb)
                        if t == 0:
                            dump_tile("rbs0", rbs[:, 0, :])
                        nc.vector.tensor_mul(oT[0:64, t, :], avA[0:64, :],
                                             rbs[:, 0, :])
                        ot_odd = oop.tile([64, QL], BF16, tag="oo")
                        nc.vector.tensor_mul(ot_odd, avB[0:64, :],
                                             rbs[:, 1, :])
                        nc.sync.dma_start(oT[64:128, t, :], ot_odd)

                    dump_tile("oT", oT[:, :, :])

                    # ====== O-proj in [q, e] + residual ======
                    for qb in range(4):
                        qsl = slice(qb * PT, (qb + 1) * PT)
                        for eh in range(2):
                            esl = slice(eh * 512, (eh + 1) * 512)
                            psf = prp.tile([PT, 512], F32, tag="pp",
                                           name=f"psf{qb}{eh}")
                            for ic in range(NE):
                                nc.tensor.matmul(
                                    psf, oT[:, ic, qsl], wo_sb[:, ic, esl],
                                    start=(ic == 0),
                                    stop=(ic == NE - 1 and "o" not in pbr))
                            if "o" in pbr:
                                nc.tensor.matmul(psf, ones_row[0:1, 0:PT],
                                                 pbr["o"][:, esl],
                                                 start=False, stop=True)
                            y_sb = yop.tile([PT, 512], F32, tag="y")
                            nc.vector.tensor_add(y_sb, psf,
                                                 x_sb[:, qb, esl])
                            nc.sync.dma_start(y_d[qsl, esl], y_sb)
                    for _pool in (yop, oop, rcp, ap, sp, bp, avp, scp):
                        _pool.release()
    return nc


def _prep_inputs(x, bias, mask, wq, bq, wk, bk, wv, bv, wo, bo, gate,
                 ln_g, ln_b):
    """Host-side folding + per-core sharding. Returns (in_maps, meta)."""
    gate = np.asarray(gate, np.float32)
    ln_g = np.asarray(ln_g, np.float32)
    ln_b = np.asarray(ln_b, np.float32)

    wqt = (np.asarray(wq).T * ln_g[:, None] * (SCALE * QSC)).astype(FP8_NP)
    wkt = (np.asarray(wk).T * ln_g[:, None] * KSC).astype(FP8_NP)
    wvt = (np.asarray(wv).T * ln_g[:, None] * VSC).astype(FP8_NP)
    wot = np.asarray(wo).T.astype(BF_NP)
    bqe = ((np.asarray(wq) @ ln_b + np.asarray(bq))
           * (SCALE * QSC)).astype(np.float32)
    bke = ((np.asarray(wk) @ ln_b + np.asarray(bk)) * KSC).astype(np.float32)
    bve = ((np.asarray(wv) @ ln_b + np.asarray(bv)) * VSC).astype(np.float32)
    boe = np.asarray(bo, np.float32)
    use_pbias = tuple(bool(np.any(b)) for b in (bqe, bke, bve, boe))

    mask = np.asarray(mask, np.int32)
    use_mask = not bool(np.all(mask == 1))

    def wfmt(w):  # [E_in, E_out] -> [128, 8, E]
        return np.ascontiguousarray(
            w.reshape(NE, PT, E).transpose(1, 0, 2))

    shared = {"wqt": wfmt(wqt), "wkt": wfmt(wkt), "wvt": wfmt(wvt),
              "wot": wfmt(wot)}
    for name, use, b in zip("qkvo", use_pbias, (bqe, bke, bve, boe)):
        if use:
            shared[f"b{name}e"] = b.reshape(1, E).astype(BF_NP)

    x = np.asarray(x, np.float32)
    bias = np.asarray(bias, np.float32)
    in_maps = []
    for c in range(NCORES):
        b_idx, qh = divmod(c, 2)
        q0 = qh * QL
        xr = np.roll(x[b_idx], -q0, axis=0)  # query block first
        m = {}
        m.update(shared)
        m["xc"] = np.ascontiguousarray(
            xr.reshape(NL, PT, L).transpose(1, 0, 2))
        bs = bias[b_idx][:, q0:q0 + QL, :]  # [H, QL, L]
        bs = np.roll(bs, -q0, axis=2)       # roll key axis
        eb = np.exp(gate[:, None, None] * bs)
        m["ebc"] = np.ascontiguousarray(eb.swapaxes(1, 2)).astype(BF_NP)
        if use_mask:
            mr = np.roll(mask[b_idx], -q0)
            kmf = (-10000.0 * (1.0 - mr.astype(np.float32)))
            m["kmc"] = np.ascontiguousarray(
                kmf.reshape(NL, PT).T).astype(np.float32)
            m["mqc"] = mr[:QL].astype(np.float32).reshape(1, QL)
        in_maps.append(m)
    return in_maps, (use_pbias, use_mask)


def kernel(**inputs):
    global LAST_RESULT
    in_maps, (use_pbias, use_mask) = _prep_inputs(**inputs)
    nc = _build_nc(use_pbias, use_mask)
    if not nc.is_finalized():
        nc.finalize()
    res = run_bass_kernel_spmd(nc, in_maps, core_ids=list(range(NCORES)))
    LAST_RESULT = res
    out = np.empty((B, L, E), np.float32)
    for c in range(NCORES):
        b_idx, qh = divmod(c, 2)
        out[b_idx, qh * QL:(qh + 1) * QL, :] = res.results[c]["yc"]
    return out


# revision 35
# speedup vs baseline: 1.2012x; 1.0688x over previous
"""BiasedMultiHeadAttention Trainium2 kernel (v5: attention-shadowed projections).

Sharding: 8 cores = (batch b, query-half qh). Each core computes the full
pipeline for its 512 query rows of batch b (K/V projections for the batch
are duplicated across the 2 cores sharing it). No collectives.

Device layout trick: per-core x rows are host-rolled so the core's query
block is always rows 0..511 -> one SPMD program for all 8 cores; bias/mask
are rolled consistently (softmax sum order irrelevant).

Math folding (host, exact):
  LN affine folded into weights; Q scaled by SCALE. The per-head gated bias
  is pre-exponentiated on host: eb = exp(gate_h * bias); the device computes
  softmax numerators as exp(logits) * eb (one vector multiply, no bias add).

Structure: the softmax exp on the Scalar engine (64 x ~1.1us) is the
irreducible critical path. Q/K/V projections (fp8 DoubleRow, 2x PE rate)
are streamed as PE filler inside the attention window so the whole kernel
rides the exp cadence: per head-pair t the c-loop runs QK -> exp -> *eb ->
AV while interleaving the projection matmuls for head-pair t+1. O-proj
(bf16) accumulates during the last pair and finishes after it.

PSUM budget (8 banks): QK ps [128,2,512] x2 bufs = 4, AV avA/avB [128,512]
= 2, projections pp [128,512] x2 bufs = 2. V is padded to 128 columns
(64 ch | ones | zeros) so AV stationaries are full 128-wide (FWL).
"""

import numpy as np
import ml_dtypes

import concourse.bass as bass
import concourse.tile as tile
import concourse.mybir as mybir
from concourse import bacc
from concourse.bass_utils import run_bass_kernel_spmd
from concourse.masks import make_identity

B, L, E, H = 4, 1024, 1024, 16
D = E // H
SCALE = D**-0.5
EPS = 1e-5
NCORES = 8
QL = 512  # query rows per core
PT = 128  # partitions
NL = L // PT  # 8 l-chunks
NE = E // PT  # 8 e-chunks
HP = H // 2  # 8 head pairs

F32 = mybir.dt.float32
BF16 = mybir.dt.bfloat16
FP8 = mybir.dt.float8e4
I32 = mybir.dt.int32
BF_NP = ml_dtypes.bfloat16
FP8_NP = ml_dtypes.float8_e4m3
# host-side up-scales keeping fp8 weights ~N(0, 0.5); undone at psum evac
QSC, KSC, VSC = 64.0, 16.0, 16.0

LAST_RESULT = None  # BassKernelResults of the most recent run (for test.py)


def _build_nc(use_pbias, use_mask, dump=()):
    """Build the single-core Bass program (same NEFF for all 8 cores)."""
    nc = bacc.Bacc("TRN2", target_bir_lowering=False, debug=False)
    dump_d = {}

    def dump_tile(name, ap):
        if name in dump:
            d = nc.dram_tensor("d_" + name, list(ap.shape), ap.dtype,
                               kind="ExternalOutput")
            nc.sync.dma_start(d[tuple(slice(None) for _ in ap.shape)], ap)
            dump_d[name] = d

    x_d = nc.dram_tensor("xc", [PT, NL, L], F32, kind="ExternalInput")
    eb_d = nc.dram_tensor("ebc", [H, L, QL], BF16, kind="ExternalInput")
    wq_d = nc.dram_tensor("wqt", [PT, NE, E], FP8, kind="ExternalInput")
    wk_d = nc.dram_tensor("wkt", [PT, NE, E], FP8, kind="ExternalInput")
    wv_d = nc.dram_tensor("wvt", [PT, NE, E], FP8, kind="ExternalInput")
    wo_d = nc.dram_tensor("wot", [PT, NE, E], BF16, kind="ExternalInput")
    pb_d = {}
    for name, use in zip("qkvo", use_pbias):
        if use:
            pb_d[name] = nc.dram_tensor(f"b{name}e", [1, E], BF16,
                                        kind="ExternalInput")
    if use_mask:
        km_d = nc.dram_tensor("kmc", [PT, NL], F32, kind="ExternalInput")
        mq_d = nc.dram_tensor("mqc", [1, QL], F32, kind="ExternalInput")
    y_d = nc.dram_tensor("yc", [QL, E], F32, kind="ExternalOutput")

    DR = mybir.MatmulPerfMode.DoubleRow
    NK2 = NE // 2  # fp8 DoubleRow contracts pairs of 128-chunks

    with tile.TileContext(nc) as tc:
        with (
            tc.tile_pool(name="persist", bufs=1) as pp,
            tc.tile_pool(name="consts", bufs=1) as cp,
        ):
            # ---- constants ----
            ident = cp.tile([PT, PT], BF16)
            make_identity(nc, ident)
            if any(use_pbias):
                ones_row = cp.tile([1, L], BF16)
                nc.vector.memset(ones_row, 1.0)
            eps_t = cp.tile([PT, 1], F32)
            nc.vector.memset(eps_t, EPS)
            if use_mask:
                km_sb = cp.tile([PT, NL], F32)
                nc.sync.dma_start(km_sb, km_d[:, :])
                mqb = cp.tile([64, QL], F32)
                nc.gpsimd.dma_start(mqb,
                                    mq_d[0:1, :].partition_broadcast(64))

            # ---- resident tensors ----
            x_sb = pp.tile([PT, NL, L], F32)
            for lt in range(NL):
                nc.sync.dma_start(x_sb[:, lt, :], x_d[:, lt, :])
            wo_sb = pp.tile([PT, NE, E], BF16)
            # K^T zero-padded per head parity (full-128 QK contracts with
            # the other head's rows zeroed; keeps FWL on the weight path)
            kTzA = pp.tile([PT, NE, L], BF16)
            kTzB = pp.tile([PT, NE, L], BF16)
            nc.gpsimd.memset(kTzA[64:128, :, :], 0.0)
            nc.gpsimd.memset(kTzB[0:64, :, :], 0.0)
            # V padded to 128 cols per head: [ch(64) | ones | junk(63)].
            # av rows 65:128 are never read, so the junk cols stay
            # uninitialized instead of paying a big gpsimd memset.
            v4 = pp.tile([PT, NL, H, PT], BF16)
            nc.vector.memset(v4[:, :, :, 64:65], 1.0)
            qT = pp.tile([PT, NE, QL], BF16)    # Q^T (scaled) [e_q, q]
            oT = pp.tile([PT, NE, QL], BF16)    # attnout^T (normalized)
            pbr = {}
            for name in pb_d:
                pbr[name] = cp.tile([1, E], BF16)
                nc.sync.dma_start(pbr[name], pb_d[name][:, :])

            with (
                tc.tile_pool(name="xnt", bufs=1) as xp,
                tc.tile_pool(name="wt", bufs=1) as wtp,
            ):
                xnT = xp.tile([PT, NE, L], FP8)  # xn^T [e, l]
                wk_sb = wtp.tile([PT, NE, E], FP8)
                nc.sync.dma_start(wk_sb, wk_d[:, :, :])
                wq_sb = wtp.tile([PT, NE, E], FP8)
                nc.sync.dma_start(wq_sb, wq_d[:, :, :])
                wv_sb = wtp.tile([PT, NE, E], FP8)
                nc.sync.dma_start(wv_sb, wv_d[:, :, :])
                nc.sync.dma_start(wo_sb, wo_d[:, :, :])

                with (
                    tc.tile_pool(name="prj", bufs=2, space="PSUM") as prp,
                ):
                    # ---------- projection helpers (filler units) --------
                    def proj_k(ot):
                        osl = slice(ot * PT, (ot + 1) * PT)
                        for nh in range(2):
                            nsl = slice(nh * 512, (nh + 1) * 512)
                            ps = prp.tile([PT, 512], F32, tag="pp",
                                          name=f"psk{ot}{nh}")
                            for k2 in range(NK2):
                                ksl = slice(2 * k2, 2 * k2 + 2)
                                yield nc.tensor.matmul(
                                    ps, wk_sb[:, ksl, osl], xnT[:, ksl, nsl],
                                    start=(k2 == 0),
                                    stop=(k2 == NK2 - 1 and "k" not in pbr),
                                    perf_mode=DR)
                            if "k" in pbr:
                                yield nc.tensor.matmul(
                                    ps, pbr["k"][:, osl], ones_row[:, 0:512],
                                    start=False, stop=True)
                            yield nc.scalar.mul(kTzA[0:64, ot, nsl],
                                                ps[0:64, :], 1.0 / KSC)
                            yield nc.vector.tensor_scalar_mul(
                                kTzB[64:128, ot, nsl], ps[64:128, :],
                                1.0 / KSC)

                    def proj_q(ot):
                        osl = slice(ot * PT, (ot + 1) * PT)
                        psq = prp.tile([PT, 512], F32, tag="pp",
                                       name=f"psq{ot}")
                        for k2 in range(NK2):
                            ksl = slice(2 * k2, 2 * k2 + 2)
                            yield nc.tensor.matmul(
                                psq, wq_sb[:, ksl, osl], xnT[:, ksl, 0:512],
                                start=(k2 == 0),
                                stop=(k2 == NK2 - 1 and "q" not in pbr),
                                perf_mode=DR)
                        if "q" in pbr:
                            yield nc.tensor.matmul(
                                psq, pbr["q"][:, osl], ones_row[:, 0:512],
                                start=False, stop=True)
                        yield nc.vector.tensor_scalar_mul(qT[:, ot, 0:512],
                                                          psq, 1.0 / QSC)

                    def proj_v(lt, vh):
                        lsl = slice(lt * PT, (lt + 1) * PT)
                        vsl = slice(vh * 512, (vh + 1) * 512)
                        psv = prp.tile([PT, 512], F32, tag="pp",
                                       name=f"psv{lt}{vh}")
                        for k2 in range(NK2):
                            ksl = slice(2 * k2, 2 * k2 + 2)
                            yield nc.tensor.matmul(
                                psv, xnT[:, ksl, lsl], wv_sb[:, ksl, vsl],
                                start=(k2 == 0),
                                stop=(k2 == NK2 - 1 and "v" not in pbr),
                                perf_mode=DR)
                        if "v" in pbr:
                            yield nc.tensor.matmul(
                                psv, ones_row[:, 0:PT], pbr["v"][:, vsl],
                                start=False, stop=True)
                        yield nc.scalar.mul(
                            v4[:, lt, vh * 8:(vh + 1) * 8, 0:64],
                            psv.rearrange("p (h d) -> p h d", h=8),
                            1.0 / VSC)

                    # ========== Phase 1: LayerNorm + transpose + V(vh0) ==
                    lp = tc.alloc_tile_pool(name="ln", bufs=3)
                    ptp = tc.alloc_tile_pool(name="pst", bufs=2,
                                             space="PSUM")
                    for lt in range(NL):
                        xr = x_sb[:, lt, :].rearrange("p (s d) -> p s d",
                                                      s=2)
                        stats = lp.tile([PT, 2, 6], F32, tag="stats")
                        for sg in range(2):
                            nc.vector.bn_stats(stats[:, sg, :], xr[:, sg, :])
                        mv = lp.tile([PT, 2], F32, tag="mv")
                        nc.vector.bn_aggr(mv, stats)
                        sd = lp.tile([PT, 1], F32, tag="sd")
                        nc.scalar.activation(
                            sd, mv[:, 1:2],
                            mybir.ActivationFunctionType.Sqrt, bias=eps_t)
                        rs = lp.tile([PT, 1], F32, tag="rs")
                        nc.vector.reciprocal(rs, sd)
                        nmr = lp.tile([PT, 1], F32, tag="nmr")  # -mu*rs
                        nc.vector.tensor_scalar(
                            out=nmr, in0=mv[:, 0:1], scalar1=rs,
                            scalar2=-1.0, op0=mybir.AluOpType.mult,
                            op1=mybir.AluOpType.mult)
                        xnb = lp.tile([PT, L], BF16, tag="xnb")
                        nc.scalar.activation(
                            xnb, x_sb[:, lt, :],
                            mybir.ActivationFunctionType.Identity,
                            bias=nmr, scale=rs)
                        for g in range(2):
                            psT = ptp.tile([PT, 512], BF16, tag="psT")
                            for j in range(4):
                                et = g * 4 + j
                                nc.tensor.transpose(
                                    psT[:, j * PT:(j + 1) * PT],
                                    xnb[:, et * PT:(et + 1) * PT], ident)
                            dst = xnT[:, g * 4:(g + 1) * 4,
                                      lt * PT:(lt + 1) * PT]
                            src = psT.rearrange("p (j l) -> p j l", j=4)
                            if g == 0:
                                nc.scalar.copy(dst, src)
                            else:
                                nc.vector.tensor_copy(dst, src)
                        # V(lt, vh=0) right behind this lt's transposes
                        for _ in proj_v(lt, 0):
                            pass

                    ptp.release()
                    lp.release()
                    scp = tc.alloc_tile_pool(name="sc", bufs=2,
                                             space="PSUM")
                    avp = tc.alloc_tile_pool(name="av", bufs=1,
                                             space="PSUM")
                    bp = tc.alloc_tile_pool(name="bias", bufs=5)
                    sp = tc.alloc_tile_pool(name="expp", bufs=3)
                    ap = tc.alloc_tile_pool(name="attn", bufs=3)
                    rcp = tc.alloc_tile_pool(name="nrm", bufs=2)
                    oop = tc.alloc_tile_pool(name="oo", bufs=2)
                    yop = tc.alloc_tile_pool(name="yo", bufs=1)

                    dump_tile("xnT", xnT[:, :, :])

                    # K/Q for head pair 0 (rest streams in the shadow)
                    for _ in proj_k(0):
                        pass
                    for _ in proj_q(0):
                        pass

                    # filler stream: K/Q for pairs 1..7, V vh=1 for all lt
                    def filler_ops():
                        for t1 in range(1, HP):
                            yield from proj_k(t1)
                            yield from proj_q(t1)
                            if t1 <= 4:
                                for lt in (2 * (t1 - 1), 2 * t1 - 1):
                                    yield from proj_v(lt, 1)
                    filler = filler_ops()
                    FILLER_PER_C = 4

                    # ================= attention ======================
                    pending = []  # deferred normalize ops (prev pair)
                    for t in range(HP):
                        hA, hB = 2 * t, 2 * t + 1
                        avA = avp.tile([PT, QL], F32, tag="avA")
                        avB = avp.tile([PT, QL], F32, tag="avB")
                        for c in range(NL):
                            csl = slice(c * PT, (c + 1) * PT)
                            ps = scp.tile([PT, 2, QL], F32, tag="ps")
                            nc.tensor.matmul(ps[:, 0, :], kTzA[:, t, csl],
                                             qT[:, t, :], start=True,
                                             stop=True)
                            nc.tensor.matmul(ps[:, 1, :], kTzB[:, t, csl],
                                             qT[:, t, :], start=True,
                                             stop=True)
                            ebt = bp.tile([PT, 2, QL], BF16, tag="ebt")
                            nc.sync.dma_start(
                                ebt, eb_d[hA:hB + 1, csl, :].rearrange(
                                    "h p q -> p h q"))
                            es = sp.tile([PT, 2, QL], BF16, tag="es")
                            kmb = km_sb[:, c:c + 1] if use_mask else 0.0
                            nc.scalar.activation(
                                es.rearrange("p h q -> p (h q)"),
                                ps.rearrange("p h q -> p (h q)"),
                                mybir.ActivationFunctionType.Exp, bias=kmb)
                            at = ap.tile([PT, 2, QL], BF16, tag="at")
                            nc.vector.tensor_mul(
                                at.rearrange("p h q -> p (h q)"),
                                es.rearrange("p h q -> p (h q)"),
                                ebt.rearrange("p h q -> p (h q)"))
                            if t == 0 and c == 0:
                                dump_tile("at0", at[:, 0, :])
                                dump_tile("at1", at[:, 1, :])
                            st, sp_ = (c == 0), (c == NL - 1)
                            nc.tensor.matmul(avA, v4[:, c, hA, :],
                                             at[:, 0, :], start=st,
                                             stop=sp_)
                            nc.tensor.matmul(avB, v4[:, c, hB, :],
                                             at[:, 1, :], start=st,
                                             stop=sp_)
                            for _ in range(FILLER_PER_C):
                                if next(filler, None) is None:
                                    break
                            if pending and c >= 1:
                                pending.pop(0)()
                        # ---- normalize part 1: evacuate av banks to
                        # SBUF right away so the next pair's AVs can
                        # claim them; recip/muls run deferred inside the
                        # next pair's stream (keeps the in-order vector
                        # queue from stalling on the gpsimd broadcast).
                        rsb = rcp.tile([65, 2, QL], F32, tag="rsb")
                        nc.vector.tensor_copy(rsb[0:65, 0, :], avA[0:65, :])
                        nc.vector.tensor_copy(rsb[0:65, 1, :], avB[0:65, :])
                        rsrow = rcp.tile([1, 2 * QL], F32, tag="rsrow",
                                         bufs=1)
                        nc.gpsimd.dma_start(
                            rsrow,
                            rsb[64:65, :, :].rearrange("p h q -> p (h q)"))
                        rbs = oop.tile([64, 2, QL], F32, tag="rbs")
                        nc.gpsimd.partition_broadcast(
                            rbs.rearrange("p h q -> p (h q)"), rsrow[0:1, :])

                        def make_norm(t, rsb, rbs):
                            def do_recip():
                                nc.vector.reciprocal_approx_fast(
                                    out=rbs.rearrange("p h q -> p (h q)"),
                                    in_=rbs.rearrange("p h q -> p (h q)"))
                                if use_mask:
                                    for hi in range(2):
                                        nc.vector.tensor_mul(
                                            rbs[:, hi, :], rbs[:, hi, :],
                                            mqb)
                            def do_muls():
                                nc.vector.tensor_mul(oT[0:64, t, :],
                                                     rsb[0:64, 0, :],
                                                     rbs[:, 0, :])
                                ot_odd = oop.tile([64, QL], BF16, tag="oo")
                                nc.vector.tensor_mul(ot_odd,
                                                     rsb[0:64, 1, :],
                                                     rbs[:, 1, :])
                                nc.sync.dma_start(oT[64:128, t, :], ot_odd)
                            return [do_recip, do_muls]

                        pending.extend(make_norm(t, rsb, rbs))
                    while pending:
                        pending.pop(0)()

                    dump_tile("oT", oT[:, :, :])

                    # ====== O-proj in [q, e] + residual ======
                    for qb in range(4):
                        qsl = slice(qb * PT, (qb + 1) * PT)
                        for eh in range(2):
                            esl = slice(eh * 512, (eh + 1) * 512)
                            psf = prp.tile([PT, 512], F32, tag="pp",
                                           name=f"psf{qb}{eh}")
                            for ic in range(NE):
                                nc.tensor.matmul(
                                    psf, oT[:, ic, qsl], wo_sb[:, ic, esl],
                                    start=(ic == 0),
                                    stop=(ic == NE - 1 and "o" not in pbr))
                            if "o" in pbr:
                                nc.tensor.matmul(psf, ones_row[0:1, 0:PT],
                                                 pbr["o"][:, esl],
                                                 start=False, stop=True)
                            y_sb = yop.tile([PT, 512], F32, tag="y")
                            nc.vector.tensor_add(y_sb, psf,
                                                 x_sb[:, qb, esl])
                            nc.sync.dma_start(y_d[qsl, esl], y_sb)
                    for _pool in (yop, oop, rcp, ap, sp, bp, avp, scp):
                        _pool.release()
    return nc


def _prep_inputs(x, bias, mask, wq, bq, wk, bk, wv, bv, wo, bo, gate,
                 ln_g, ln_b):
    """Host-side folding + per-core sharding. Returns (in_maps, meta)."""
    gate = np.asarray(gate, np.float32)
    ln_g = np.asarray(ln_g, np.float32)
    ln_b = np.asarray(ln_b, np.float32)

    wqt = (np.asarray(wq).T * ln_g[:, None] * (SCALE * QSC)).astype(FP8_NP)
    wkt = (np.asarray(wk).T * ln_g[:, None] * KSC).astype(FP8_NP)
    wvt = (np.asarray(wv).T * ln_g[:, None] * VSC).astype(FP8_NP)
    wot = np.asarray(wo).T.astype(BF_NP)
    bqe = ((np.asarray(wq) @ ln_b + np.asarray(bq))
           * (SCALE * QSC)).astype(np.float32)
    bke = ((np.asarray(wk) @ ln_b + np.asarray(bk)) * KSC).astype(np.float32)
    bve = ((np.asarray(wv) @ ln_b + np.asarray(bv)) * VSC).astype(np.float32)
    boe = np.asarray(bo, np.float32)
    use_pbias = tuple(bool(np.any(b)) for b in (bqe, bke, bve, boe))

    mask = np.asarray(mask, np.int32)
    use_mask = not bool(np.all(mask == 1))

    def wfmt(w):  # [E_in, E_out] -> [128, 8, E]
        return np.ascontiguousarray(
            w.reshape(NE, PT, E).transpose(1, 0, 2))

    shared = {"wqt": wfmt(wqt), "wkt": wfmt(wkt), "wvt": wfmt(wvt),
              "wot": wfmt(wot)}
    for name, use, b in zip("qkvo", use_pbias, (bqe, bke, bve, boe)):
        if use:
            shared[f"b{name}e"] = b.reshape(1, E).astype(BF_NP)

    x = np.asarray(x, np.float32)
    bias = np.asarray(bias, np.float32)
    in_maps = []
    for c in range(NCORES):
        b_idx, qh = divmod(c, 2)
        q0 = qh * QL
        xr = np.roll(x[b_idx], -q0, axis=0)  # query block first
        m = {}
        m.update(shared)
        m["xc"] = np.ascontiguousarray(
            xr.reshape(NL, PT, L).transpose(1, 0, 2))
        bs = bias[b_idx][:, q0:q0 + QL, :]  # [H, QL, L]
        bs = np.roll(bs, -q0, axis=2)       # roll key axis
        eb = np.exp(gate[:, None, None] * bs)
        m["ebc"] = np.ascontiguousarray(eb.swapaxes(1, 2)).astype(BF_NP)
        if use_mask:
            mr = np.roll(mask[b_idx], -q0)
            kmf = (-10000.0 * (1.0 - mr.astype(np.float32)))
            m["kmc"] = np.ascontiguousarray(
                kmf.reshape(NL, PT).T).astype(np.float32)
            m["mqc"] = mr[:QL].astype(np.float32).reshape(1, QL)
        in_maps.append(m)
    return in_maps, (use_pbias, use_mask)


def kernel(**inputs):
    global LAST_RESULT
    in_maps, (use_pbias, use_mask) = _prep_inputs(**inputs)
    nc = _build_nc(use_pbias, use_mask)
    if not nc.is_finalized():
        nc.finalize()
    res = run_bass_kernel_spmd(nc, in_maps, core_ids=list(range(NCORES)))
    LAST_RESULT = res
    out = np.empty((B, L, E), np.float32)
    for c in range(NCORES):
        b_idx, qh = divmod(c, 2)
        out[b_idx, qh * QL:(qh + 1) * QL, :] = res.results[c]["yc"]
    return out
